# revision 1
# baseline (speedup 1.0000x reference)
"""Trainium2 Bass kernel for sparse 3D voxel convolution (e3nn-style, 5^3 taps).

Sharding: data-parallel over the N=200000 sparse voxels. Voxels are sorted by
x-plane and split into 8 contiguous slabs of 25000 destination voxels; each
core gets a local feature table holding its slab plus the +/-2-plane halo
(<32k rows, so int16 gather indices work). The tiny kernel-generation weights
(8x2304) and residual Linear weights are folded on host into 125 tap matrices
K[80,80] (residual into the center tap), replicated to all cores.

Device pipeline per core:
  - center tap (every voxel, src==dst): direct HWDGE DMA load of 128-row
    blocks -> PE transpose -> matmul(lhsT=X^T, rhs=K62) -> direct store.
  - 124 sparse taps: host-planned pair stream (columns of 128 pairs, padded
    per tap, identical column->tap schedule on all cores). dma_gather
    (SWDGE custom op, int16 local ids) -> PE transpose -> matmul ->
    dma_scatter_add into the output table. Scatter instructions never cross
    tap boundaries (destinations are unique within a tap; duplicates within
    one scatter instruction lose updates on HW). Pad slots gather row 0 and
    scatter into a trash row past the real output rows.
"""

import sys
import types

import numpy as np

NRB = 8
RAD = 2.5
GRID = 192
N = 200000
DIM = 80
EP = 128                       # padded feature row (512B) for dma_gather
ALPHA = 1.0 / np.sqrt(48.0)
N_CORES = 8
N_LOC = N // N_CORES           # 25000 dst voxels per core
CJ = 16                        # columns per center group (direct DMA path)
NCEN = 13 * CJ                 # 208 center columns -> covers rows 0..26623
CEN_ROWS = NCEN * 128          # 26624
TRASH = CEN_ROWS               # scatter trash row
NT = CEN_ROWS + 128            # out table rows
JMAX = 8                       # max columns per gather/scatter instruction
                               # (custom-op ucode fails above 1024 indices)
SUB = 4                        # columns per PSUM bank group

_ax = np.arange(-2.0, 3.0, dtype=np.float32)
LATTICE = np.stack(np.meshgrid(_ax, _ax, _ax, indexing="ij"), -1)
PERM = np.arange(125).reshape(5, 5, 5).transpose(2, 1, 0).reshape(-1)
OFFS = LATTICE.reshape(-1, 3).astype(np.int32)[PERM]
CENTER_TAP = 62


def _radial_emb():
    d = np.linalg.norm(LATTICE, axis=-1)
    centers = np.linspace(0.0, RAD, NRB + 2)[1:-1]
    step = centers[1] - centers[0]
    t = (d[..., None] - centers) / step
    inside = np.abs(t) < 1.0
    safe = np.where(inside, 1.0 - t * t, 1.0)
    return (1.14136 * np.exp(2.0) * np.where(inside, np.exp(-2.0 / safe), 0.0)).astype(
        np.float32
    )


EMB = _radial_emb().reshape(-1, NRB)


def _sph():
    n = np.linalg.norm(LATTICE, axis=-1, keepdims=True)
    u = np.where(n > 0, LATTICE / np.maximum(n, 1e-9), 0.0)
    return np.concatenate([np.ones_like(n), np.sqrt(3.0) * u], -1).astype(np.float32)


SH = _sph().reshape(-1, 4)


def make_kernel_np(weight):
    w = (EMB @ weight.astype(np.float32)) / 125.0
    w1 = w[:, :1024].reshape(125, 32, 32)
    w2 = w[:, 1024:1536].reshape(125, 32, 16)
    w3 = w[:, 1536:1792].reshape(125, 16, 16)
    w4 = w[:, 1792:].reshape(125, 16, 32)
    s0 = SH[:, 0]
    v = SH[:, 1:]
    eye3 = np.eye(3, dtype=w.dtype)
    K00 = ALPHA * w1 * s0[:, None, None]
    K01 = ALPHA * np.einsum("pik,pm->pikm", w2, v).reshape(125, 32, 48)
    K11 = ALPHA * np.einsum(
        "pik,mn->pimkn", w3 * s0[:, None, None], eye3
    ).reshape(125, 48, 48)
    K10 = (ALPHA / np.sqrt(3.0)) * np.einsum("pik,pm->pimk", w4, v).reshape(125, 48, 32)
    K = np.concatenate(
        [np.concatenate([K00, K01], 2), np.concatenate([K10, K11], 2)], 1
    )
    return K[PERM]


def w_sc_embed(w_sc0, w_sc1):
    W = np.zeros((80, 80), np.float32)
    W[:32, :32] = w_sc0 / np.sqrt(32.0)
    blk = np.zeros((48, 48), np.float32)
    for m in range(3):
        blk[m::3, m::3] = w_sc1 / np.sqrt(16.0)
    W[32:, 32:] = blk
    return W


def build_pairs(coords):
    idx_vol = np.full(GRID * GRID * GRID, -1, np.int32)
    lin = (coords[:, 0].astype(np.int64) * GRID + coords[:, 1]) * GRID + coords[:, 2]
    idx_vol[lin] = np.arange(N, dtype=np.int32)
    all_i = np.arange(N, dtype=np.int32)
    dsts, srcs = [], []
    for t in range(125):
        if t == CENTER_TAP:
            dsts.append(None)
            srcs.append(None)
            continue
        c = coords + OFFS[t]
        ok = np.all((c >= 0) & (c < GRID), axis=1)
        cl = (c[:, 0].astype(np.int64) * GRID + c[:, 1]) * GRID + c[:, 2]
        cl = np.clip(cl, 0, GRID**3 - 1)
        nb = idx_vol[cl]
        valid = ok & (nb >= 0)
        dsts.append(all_i[valid])
        srcs.append(nb[valid])
    return dsts, srcs


def wrap16(a):
    """Token stream [n] -> [128, n//16] int16 (16-partition wrap, 8x replicated)."""
    n = a.shape[0]
    w = a.reshape(n // 16, 16).T
    return np.ascontiguousarray(np.tile(w, (8, 1)).astype(np.int16))


def build_plan(feats, coords):
    """Returns (feats_loc [8,SRC_ROWS,EP], gidx_w [8,128,8W], sidx_w [8,128,8W],
    groups, order) where groups is a list of
    (ncols, [(tap, col_lo, col_hi), ...]) shared by all cores."""
    order = np.argsort(coords[:, 0], kind="stable").astype(np.int32)
    pos = np.empty(N, np.int32)
    pos[order] = np.arange(N, dtype=np.int32)
    core_of = pos // N_LOC
    loc_dst = pos % N_LOC

    dsts, srcs = build_pairs(coords)
    taps = [t for t in range(125) if t != CENTER_TAP]

    # per (core, tap) local pair lists
    per_core = [[None] * 125 for _ in range(N_CORES)]
    for t in taps:
        d, s = dsts[t], srcs[t]
        cd = core_of[d]
        for c in range(N_CORES):
            m = cd == c
            dl = loc_dst[d[m]]
            sg = s[m]
            o = np.argsort(dl, kind="stable")
            per_core[c][t] = (dl[o], sg[o])

    # local src tables
    SRC_MIN = CEN_ROWS
    glob2loc = np.full((N_CORES, N), -1, np.int32)
    extras = []
    for c in range(N_CORES):
        dg = order[c * N_LOC : (c + 1) * N_LOC]
        glob2loc[c, dg] = np.arange(N_LOC, dtype=np.int32)
        need = np.unique(np.concatenate([per_core[c][t][1] for t in taps]))
        ex = need[glob2loc[c, need] < 0]
        glob2loc[c, ex] = N_LOC + np.arange(len(ex), dtype=np.int32)
        extras.append(ex)
    n_src = [N_LOC + len(e) for e in extras]
    SRC_ROWS = max(SRC_MIN, max(n_src))
    assert SRC_ROWS <= 32767, n_src
    feats_loc = np.zeros((N_CORES, SRC_ROWS, EP), np.float32)
    for c in range(N_CORES):
        dg = order[c * N_LOC : (c + 1) * N_LOC]
        feats_loc[c, :N_LOC, :DIM] = feats[dg]
        feats_loc[c, N_LOC : n_src[c], :DIM] = feats[extras[c]]

    # columns per tap (max over cores); fixed JMAX-column windows, taps may
    # span windows (scatter slices stay within one tap -> unique dsts)
    w_t = {
        t: max(
            1,
            max((len(per_core[c][t][0]) + 127) // 128 for c in range(N_CORES)),
        )
        for t in taps
    }
    col_tap = []
    for t in taps:
        col_tap += [t] * w_t[t]
    W = sum(w_t.values())
    groups = []
    col = 0
    while col < W:
        wg = min(JMAX, W - col)
        runs = []
        a = 0
        while a < wg:
            t = col_tap[col + a]
            b = a
            while b < wg and col_tap[col + b] == t:
                b += 1
            runs.append((t, a, b))
            a = b
        groups.append((wg, runs))
        col += wg
    gidx = np.zeros((N_CORES, W * 128), np.int32)
    sidx = np.full((N_CORES, W * 128), TRASH, np.int32)
    col = 0
    for t in taps:
        for c in range(N_CORES):
            dl, sg = per_core[c][t]
            m = len(dl)
            a = col * 128
            gidx[c, a : a + m] = glob2loc[c, sg]
            sidx[c, a : a + m] = dl
        col += w_t[t]
    assert col == W

    # token i within its 128-col block: column-major packing (token = c*128+p)
    gidx_w = np.stack([wrap16(gidx[c]) for c in range(N_CORES)])
    sidx_w = np.stack([wrap16(sidx[c]) for c in range(N_CORES)])
    return feats_loc, gidx_w, sidx_w, groups, order, SRC_ROWS


def _install_axon_profile_hook():
    try:
        import antenv

        if "antenv.axon_hooks" not in sys.modules:
            mod = types.ModuleType("antenv.axon_hooks")
            hook = [None]
            mod.set_axon_ntff_profile_hook = lambda h: hook.__setitem__(0, h)
            mod.get_axon_ntff_profile_hook = lambda: hook[0]
            sys.modules["antenv.axon_hooks"] = mod
            antenv.axon_hooks = mod
        from antenv.axon_hooks import (
            get_axon_ntff_profile_hook,
            set_axon_ntff_profile_hook,
        )

        if get_axon_ntff_profile_hook() is None:
            from trn_agent_boot.trn_boot import _ntff_profile_via_ctypes

            set_axon_ntff_profile_hook(
                _ntff_profile_via_ctypes("/opt/axon/libaxon_pjrt.so")
            )
    except Exception:
        pass


def build_program(groups, SRC_ROWS, W, do_center=True, ngroups=None, do_scatter=True):
    import os
    n_queues = int(os.environ.get("K_QUEUES", "4"))
    n_tables = int(os.environ.get("K_TABLES", "2"))
    import concourse.bacc as bacc
    import concourse.mybir as mybir
    import concourse.tile as tile
    from concourse.masks import make_identity

    nc = bacc.Bacc(
        "TRN2", num_devices=N_CORES, debug=False, target_bir_lowering=False,
        num_swdge_queues=n_queues,
    )
    f32 = mybir.dt.float32
    i16 = mybir.dt.int16

    feats_d = nc.dram_tensor("feats_loc", [SRC_ROWS, EP], f32, kind="ExternalInput").ap()
    ktaps_d = nc.dram_tensor("ktaps", [80, 125 * 80], f32, kind="ExternalInput").ap()
    gidx_d = nc.dram_tensor("gidx", [128, 8 * W], i16, kind="ExternalInput").ap()
    sidx_d = nc.dram_tensor("sidx", [128, 8 * W], i16, kind="ExternalInput").ap()
    out_d = nc.dram_tensor("out", [NT, EP], f32, kind="ExternalOutput").ap()
    tbl = [
        nc.dram_tensor(f"tbl{i}", [NT, EP], f32, kind="ExternalOutput").ap()
        for i in range(n_tables)
    ]

    with tile.TileContext(nc) as tc:
        with (
            tc.tile_pool(name="const", bufs=1) as cpool,
            tc.tile_pool(name="gath", bufs=3) as gpool,
            tc.tile_pool(name="xts", bufs=4) as xpool,
            tc.tile_pool(name="ysb", bufs=3) as ypool,
            tc.tile_pool(name="xtp", bufs=4, space="PSUM") as xppool,
            tc.tile_pool(name="ypp", bufs=4, space="PSUM") as yppool,
        ):
            ident = cpool.tile([128, 128], f32)
            make_identity(nc, ident[:])
            ksb = cpool.tile([80, 125 * 80], f32)
            nc.sync.dma_start(out=ksb[:], in_=ktaps_d[:])
            gsb = cpool.tile([128, 8 * W], i16)
            nc.sync.dma_start(out=gsb[:], in_=gidx_d[:])
            ssb = cpool.tile([128, 8 * W], i16)
            nc.sync.dma_start(out=ssb[:], in_=sidx_d[:])

            def compute_block(G, wg, col_taps, Y):
                """G [128, wg, EP] gathered block -> Y [128, wg, DIM]."""
                for sb in range((wg + SUB - 1) // SUB):
                    k0 = sb * SUB
                    kn = min(SUB, wg - k0)
                    xt_ps = xppool.tile([80, kn, 128], f32, tag="xtp")
                    for k in range(kn):
                        nc.tensor.transpose(
                            out=xt_ps[:, k, :],
                            in_=G[:, k0 + k, :DIM],
                            identity=ident[:],
                        )
                    xt_sb = xpool.tile([80, kn, 128], f32, tag="xts")
                    nc.vector.tensor_copy(out=xt_sb[:], in_=xt_ps[:])
                    y_ps = yppool.tile([128, kn, DIM], f32, tag="ypp")
                    for k in range(kn):
                        t = col_taps[k0 + k]
                        nc.tensor.matmul(
                            out=y_ps[:, k, :],
                            lhsT=xt_sb[:, k, :],
                            rhs=ksb[:, t * 80 : (t + 1) * 80],
                            start=True,
                            stop=True,
                        )
                    nc.vector.tensor_copy(out=Y[:, k0 : k0 + kn, :], in_=y_ps[:])

            # ---- center tap: direct DMA both ways --------------------------
            for g in range(NCEN // CJ if do_center else 0):
                r0 = g * CJ * 128
                G = gpool.tile([128, CJ, EP], f32, tag="G")
                nc.sync.dma_start(
                    out=G[:],
                    in_=feats_d[r0 : r0 + CJ * 128, :].rearrange(
                        "(c p) f -> p c f", p=128
                    ),
                )
                Y = ypool.tile([128, CJ, DIM], f32, tag="Y")
                compute_block(G, CJ, [CENTER_TAP] * CJ, Y)
                nc.sync.dma_start(
                    out=out_d[r0 : r0 + CJ * 128, :DIM].rearrange(
                        "(c p) f -> p c f", p=128
                    ),
                    in_=Y[:],
                )

            # ---- sparse taps: dma_gather + per-tap dma_scatter_add ---------
            n_scat = 0
            col = 0
            for gi_, (wg, gtaps) in enumerate(groups):
                if ngroups is not None and gi_ >= ngroups:
                    break
                i0 = col * 8
                G = gpool.tile([128, wg, EP], f32, tag="G")
                nc.gpsimd.dma_gather(
                    out_ap=G[:],
                    in_ap=feats_d[:],
                    idxs_ap=gsb[:, i0 : i0 + 8 * wg],
                    num_idxs=wg * 128,
                    num_idxs_reg=wg * 128,
                    elem_size=EP,
                    queue_num=(2 * (gi_ % 2)) % n_queues,
                )
                col_taps = []
                for t, a, b in gtaps:
                    col_taps += [t] * (b - a)
                Y = ypool.tile([128, wg, DIM], f32, tag="Y")
                compute_block(G, wg, col_taps, Y)
                for t, a, b in (gtaps if do_scatter else []):
                    nc.gpsimd.dma_scatter_add(
                        out_ap=tbl[n_scat % n_tables][:, :DIM],
                        in_ap=Y[:, a:b, :],
                        idxs_ap=ssb[:, i0 + 8 * a : i0 + 8 * b],
                        num_idxs=(b - a) * 128,
                        num_idxs_reg=(b - a) * 128,
                        elem_size=DIM,
                        elem_step=EP,
                        queue_num=(1 + 2 * (n_scat % n_tables)) % n_queues if n_queues > 1 else 0,
                    )
                    n_scat += 1
                col += wg
    print("tile build done", file=sys.stderr)
    nc.compile()
    print("bacc compile done", file=sys.stderr)
    return nc


_LAST = {"exec_time_ns": None, "results": None}


def kernel(feats, weight, w_sc0, w_sc1, coords):
    feats = np.ascontiguousarray(np.asarray(feats, np.float32))
    weight = np.asarray(weight, np.float32)
    w_sc0 = np.asarray(w_sc0, np.float32)
    w_sc1 = np.asarray(w_sc1, np.float32)
    coords = np.asarray(coords, np.int32)

    K = make_kernel_np(weight)
    K[CENTER_TAP] = K[CENTER_TAP] + w_sc_embed(w_sc0, w_sc1)
    ktaps = np.ascontiguousarray(K.transpose(1, 0, 2).reshape(80, 125 * 80))

    feats_loc, gidx_w, sidx_w, groups, order, SRC_ROWS = build_plan(feats, coords)
    W = gidx_w.shape[2] // 8
    print(f"plan: W={W} groups={len(groups)} SRC_ROWS={SRC_ROWS}", file=sys.stderr)

    _install_axon_profile_hook()
    from concourse.bass_utils import run_bass_kernel_spmd

    nc = build_program(groups, SRC_ROWS, W)
    in_maps = [
        {
            "feats_loc": feats_loc[c],
            "ktaps": ktaps,
            "gidx": gidx_w[c],
            "sidx": sidx_w[c],
        }
        for c in range(N_CORES)
    ]
    import os

    trace = os.environ.get("BASS_KERNEL_TRACE", "0") == "1"
    import time as _time

    res = None
    last_exc = None
    for attempt in range(4):
        try:
            res = run_bass_kernel_spmd(
                nc,
                in_maps,
                core_ids=list(range(N_CORES)),
                trace=trace and attempt == 0,
            )
            break
        except Exception as e:  # device flake: retry, last attempts untraced
            last_exc = e
            print(f"run attempt {attempt} failed: {e}", file=sys.stderr)
            _time.sleep(3.0)
    if res is None:
        raise last_exc
    print("hw run done", file=sys.stderr)
    _LAST["exec_time_ns"] = res.exec_time_ns
    _LAST["results"] = res
    out = np.empty((N, DIM), np.float32)
    for c in range(N_CORES):
        out[order[c * N_LOC : (c + 1) * N_LOC]] = np.asarray(
            res.results[c]["out"]
        )[:N_LOC, :DIM]
    return out



# revision 2
# speedup vs baseline: 1.0723x; 1.0723x over previous
"""Trainium2 Bass kernel V3 for sparse 3D voxel convolution (e3nn-style, 5^3 taps).

Sharding: data-parallel over the N=200000 sparse voxels, sorted by x-plane and
split into 8 contiguous slabs of 25000 destination voxels; each core holds a
local bf16 feature table (slab + halo, <32k rows, int16 gather ids).

Single-phase per-tap pipeline per core:
  - center tap + residual: the slab's features live transposed in SBUF
    ([feat, dst] bf16 strip); one matmul per 128-dst block against the
    center kernel accumulates in PSUM and stores contiguous f32 output rows.
  - 124 sparse taps, one gather + one scatter-add per tap (pair lists padded
    to the max count over cores; pads gather row 0 and scatter into a trash
    row): bf16 dma_gather (256B rows) -> PE transpose (bf16 identity) ->
    matmul against the tap kernel -> bf16 dma_scatter_add (160B payload,
    256B row pitch) into one of 4 bf16 tables, table == queue so RMW stays
    ordered. Destinations are unique within a tap, so no updates are lost.

Host sums out + the 4 tables (the conv part is ~1% of the residual, bf16
accumulation is far inside the tolerance).
"""

import sys
import types

import numpy as np
import ml_dtypes

NRB = 8
RAD = 2.5
GRID = 192
N = 200000
DIM = 80
ALPHA = 1.0 / np.sqrt(48.0)
N_CORES = 8
N_LOC = N // N_CORES            # 25000 dst voxels per core
NBLK = (N_LOC + 127) // 128     # 196 out blocks
NT = NBLK * 128                 # 25088 out rows
TRASH = NT                      # scatter pad row (tables have NT+1... rows)
NTT = NT + 128                  # table rows incl trash
SUB = 4                         # columns per PSUM tile
N_TBL = 4                       # scatter tables == queues

_ax = np.arange(-2.0, 3.0, dtype=np.float32)
LATTICE = np.stack(np.meshgrid(_ax, _ax, _ax, indexing="ij"), -1)
PERM = np.arange(125).reshape(5, 5, 5).transpose(2, 1, 0).reshape(-1)
OFFS = LATTICE.reshape(-1, 3).astype(np.int32)[PERM]
CENTER_TAP = 62
TAPS = [t for t in range(125) if t != CENTER_TAP]


def _radial_emb():
    d = np.linalg.norm(LATTICE, axis=-1)
    centers = np.linspace(0.0, RAD, NRB + 2)[1:-1]
    step = centers[1] - centers[0]
    t = (d[..., None] - centers) / step
    inside = np.abs(t) < 1.0
    safe = np.where(inside, 1.0 - t * t, 1.0)
    return (1.14136 * np.exp(2.0) * np.where(inside, np.exp(-2.0 / safe), 0.0)).astype(
        np.float32
    )


EMB = _radial_emb().reshape(-1, NRB)


def _sph():
    n = np.linalg.norm(LATTICE, axis=-1, keepdims=True)
    u = np.where(n > 0, LATTICE / np.maximum(n, 1e-9), 0.0)
    return np.concatenate([np.ones_like(n), np.sqrt(3.0) * u], -1).astype(np.float32)


SH = _sph().reshape(-1, 4)


def make_kernel_np(weight):
    w = (EMB @ weight.astype(np.float32)) / 125.0
    w1 = w[:, :1024].reshape(125, 32, 32)
    w2 = w[:, 1024:1536].reshape(125, 32, 16)
    w3 = w[:, 1536:1792].reshape(125, 16, 16)
    w4 = w[:, 1792:].reshape(125, 16, 32)
    s0 = SH[:, 0]
    v = SH[:, 1:]
    eye3 = np.eye(3, dtype=w.dtype)
    K00 = ALPHA * w1 * s0[:, None, None]
    K01 = ALPHA * np.einsum("pik,pm->pikm", w2, v).reshape(125, 32, 48)
    K11 = ALPHA * np.einsum(
        "pik,mn->pimkn", w3 * s0[:, None, None], eye3
    ).reshape(125, 48, 48)
    K10 = (ALPHA / np.sqrt(3.0)) * np.einsum("pik,pm->pimk", w4, v).reshape(125, 48, 32)
    K = np.concatenate(
        [np.concatenate([K00, K01], 2), np.concatenate([K10, K11], 2)], 1
    )
    return K[PERM]


def w_sc_embed(w_sc0, w_sc1):
    W = np.zeros((80, 80), np.float32)
    W[:32, :32] = w_sc0 / np.sqrt(32.0)
    blk = np.zeros((48, 48), np.float32)
    for m in range(3):
        blk[m::3, m::3] = w_sc1 / np.sqrt(16.0)
    W[32:, 32:] = blk
    return W


def build_pairs(coords):
    idx_vol = np.full(GRID * GRID * GRID, -1, np.int32)
    lin = (coords[:, 0].astype(np.int64) * GRID + coords[:, 1]) * GRID + coords[:, 2]
    idx_vol[lin] = np.arange(N, dtype=np.int32)
    all_i = np.arange(N, dtype=np.int32)
    dsts, srcs = [], []
    for t in range(125):
        if t == CENTER_TAP:
            dsts.append(None)
            srcs.append(None)
            continue
        c = coords + OFFS[t]
        ok = np.all((c >= 0) & (c < GRID), axis=1)
        cl = (c[:, 0].astype(np.int64) * GRID + c[:, 1]) * GRID + c[:, 2]
        cl = np.clip(cl, 0, GRID**3 - 1)
        nb = idx_vol[cl]
        valid = ok & (nb >= 0)
        dsts.append(all_i[valid])
        srcs.append(nb[valid])
    return dsts, srcs


def wrap16(a):
    """Token stream [n] -> [128, n//16] int16 (16-partition wrap, 8x replicated)."""
    n = a.shape[0]
    w = a.reshape(n // 16, 16).T
    return np.ascontiguousarray(np.tile(w, (8, 1)).astype(np.int16))


def build_plan(feats, coords):
    order = np.argsort(coords[:, 0], kind="stable").astype(np.int32)
    pos = np.empty(N, np.int32)
    pos[order] = np.arange(N, dtype=np.int32)
    core_of = pos // N_LOC
    loc_dst = pos % N_LOC

    dsts, srcs = build_pairs(coords)

    per_core = [[None] * 125 for _ in range(N_CORES)]
    for t in TAPS:
        d, s = dsts[t], srcs[t]
        cd = core_of[d]
        for c in range(N_CORES):
            m = cd == c
            dl = loc_dst[d[m]]
            sg = s[m]
            o = np.argsort(dl, kind="stable")
            per_core[c][t] = (dl[o], sg[o])

    glob2loc = np.full((N_CORES, N), -1, np.int32)
    extras = []
    for c in range(N_CORES):
        dg = order[c * N_LOC : (c + 1) * N_LOC]
        glob2loc[c, dg] = np.arange(N_LOC, dtype=np.int32)
        need = np.unique(np.concatenate([per_core[c][t][1] for t in TAPS]))
        ex = need[glob2loc[c, need] < 0]
        glob2loc[c, ex] = N_LOC + np.arange(len(ex), dtype=np.int32)
        extras.append(ex)
    n_src = [N_LOC + len(e) for e in extras]
    SRC_ROWS = max(n_src)
    assert SRC_ROWS <= 32600, n_src
    feats_loc = np.zeros((N_CORES, SRC_ROWS, 128), ml_dtypes.bfloat16)
    feats_T = np.zeros((N_CORES, 128, NT), ml_dtypes.bfloat16)
    fb = feats.astype(ml_dtypes.bfloat16)
    for c in range(N_CORES):
        dg = order[c * N_LOC : (c + 1) * N_LOC]
        feats_loc[c, :N_LOC, :DIM] = fb[dg]
        feats_loc[c, N_LOC : n_src[c], :DIM] = fb[extras[c]]
        feats_T[c, :DIM, :N_LOC] = fb[dg].T

    # segments: one per tap, split by dst-halves while too wide for one op
    segments = []  # (tap, dst_lo, dst_hi, w)
    stack = [(t, 0, N_LOC) for t in TAPS]
    while stack:
        t, lo, hi = stack.pop(0)
        mx = 0
        for c in range(N_CORES):
            dl, _ = per_core[c][t]
            mx = max(mx, int(np.sum((dl >= lo) & (dl < hi))))
        w = max(1, (mx + 127) // 128)
        if w > 8:
            mid = (lo + hi) // 2
            stack = [(t, lo, mid), (t, mid, hi)] + stack
        else:
            segments.append((t, lo, hi, w))
    W = sum(s[3] for s in segments)

    gidx = np.zeros((N_CORES, W * 128), np.int32)
    sidx = np.full((N_CORES, W * 128), TRASH, np.int32)
    col = 0
    seg_cols = []
    for (t, lo, hi, w) in segments:
        seg_cols.append(col)
        for c in range(N_CORES):
            dl, sg = per_core[c][t]
            m = (dl >= lo) & (dl < hi)
            dls = dl[m]
            lids = glob2loc[c, sg[m]]
            n = len(dls)
            base = col * 128
            gidx[c, base : base + n] = lids
            sidx[c, base : base + n] = dls
        col += w
    assert col == W

    gidx_w = np.stack([wrap16(gidx[c]) for c in range(N_CORES)])
    sidx_w = np.stack([wrap16(sidx[c]) for c in range(N_CORES)])
    return feats_loc, feats_T, gidx_w, sidx_w, segments, seg_cols, W, order, SRC_ROWS


def _install_axon_profile_hook():
    try:
        import antenv

        if "antenv.axon_hooks" not in sys.modules:
            mod = types.ModuleType("antenv.axon_hooks")
            hook = [None]
            mod.set_axon_ntff_profile_hook = lambda h: hook.__setitem__(0, h)
            mod.get_axon_ntff_profile_hook = lambda: hook[0]
            sys.modules["antenv.axon_hooks"] = mod
            antenv.axon_hooks = mod
        from antenv.axon_hooks import (
            get_axon_ntff_profile_hook,
            set_axon_ntff_profile_hook,
        )

        if get_axon_ntff_profile_hook() is None:
            from trn_agent_boot.trn_boot import _ntff_profile_via_ctypes

            set_axon_ntff_profile_hook(
                _ntff_profile_via_ctypes("/opt/axon/libaxon_pjrt.so")
            )
    except Exception:
        pass


def build_program(segments, seg_cols, W, SRC_ROWS):
    import concourse.bacc as bacc
    import concourse.mybir as mybir
    import concourse.tile as tile
    from concourse.masks import make_identity

    nc = bacc.Bacc(
        "TRN2", num_devices=N_CORES, debug=False, target_bir_lowering=False,
        num_swdge_queues=4,
    )
    f32 = mybir.dt.float32
    bf16 = mybir.dt.bfloat16
    i16 = mybir.dt.int16

    feats_d = nc.dram_tensor("feats_loc", [SRC_ROWS, 128], bf16, kind="ExternalInput").ap()
    featsT_d = nc.dram_tensor("feats_T", [128, NT], bf16, kind="ExternalInput").ap()
    ktaps_d = nc.dram_tensor("ktaps", [128, 125 * 80], bf16, kind="ExternalInput").ap()
    gidx_d = nc.dram_tensor("gidx", [128, 8 * W], i16, kind="ExternalInput").ap()
    sidx_d = nc.dram_tensor("sidx", [128, 8 * W], i16, kind="ExternalInput").ap()
    out_d = nc.dram_tensor("out", [NT, 80], f32, kind="ExternalOutput").ap()
    tbl = [
        nc.dram_tensor(f"tbl{i}", [NTT, 128], bf16, kind="ExternalOutput").ap()
        for i in range(N_TBL)
    ]

    with tile.TileContext(nc) as tc:
        with (
            tc.tile_pool(name="const", bufs=1) as cpool,
            tc.tile_pool(name="gath", bufs=4) as gpool,
            tc.tile_pool(name="xsb", bufs=4) as xpool,
            tc.tile_pool(name="ysb", bufs=3) as ypool,
            tc.tile_pool(name="osb", bufs=3) as opool,
            tc.tile_pool(name="psX", bufs=3, space="PSUM") as psX,
            tc.tile_pool(name="psY", bufs=3, space="PSUM") as psY,
            tc.tile_pool(name="psB", bufs=2, space="PSUM") as psB,
        ):
            ident = cpool.tile([128, 128], bf16)
            make_identity(nc, ident[:])
            ksb = cpool.tile([128, 125 * 80], bf16)
            nc.sync.dma_start(out=ksb[:], in_=ktaps_d[:])
            ftsb = cpool.tile([128, NT], bf16)
            nc.sync.dma_start(out=ftsb[:], in_=featsT_d[:])
            gsb = cpool.tile([128, 8 * W], i16)
            nc.sync.dma_start(out=gsb[:], in_=gidx_d[:])
            ssb = cpool.tile([128, 8 * W], i16)
            nc.sync.dma_start(out=ssb[:], in_=sidx_d[:])

            cp_rr = [0]

            def do_copy(out, in_):
                cp_rr[0] += 1
                if cp_rr[0] % 2:
                    nc.scalar.copy(out=out, in_=in_)
                else:
                    nc.vector.tensor_copy(out=out, in_=in_)

            def emit_center(b):
                ps = psB.tile([128, 80], f32, tag="ops")
                d0 = b * 128
                nc.tensor.matmul(
                    out=ps[:],
                    lhsT=ftsb[:, d0 : d0 + 128],
                    rhs=ksb[:, CENTER_TAP * 80 : (CENTER_TAP + 1) * 80],
                    start=True,
                    stop=True,
                )
                ob = opool.tile([128, 80], f32, tag="ob")
                do_copy(ob[:], ps[:])
                nc.sync.dma_start(out=out_d[d0 : d0 + 128, :], in_=ob[:])

            def emit_segment(i):
                t, lo, hi, w = segments[i]
                c0 = seg_cols[i]
                ntok = w * 128
                q = i % 4
                gt = gpool.tile([128, w, 128], bf16, tag="gt")
                nc.gpsimd.dma_gather(
                    out_ap=gt[:],
                    in_ap=feats_d[:],
                    idxs_ap=gsb[:, c0 * 8 : c0 * 8 + 8 * w],
                    num_idxs=ntok,
                    num_idxs_reg=ntok,
                    elem_size=128,
                    queue_num=q,
                )
                ysb = ypool.tile([128, w, 80], bf16, tag="ysb")
                for k0 in range(0, w, SUB):
                    kn = min(SUB, w - k0)
                    xps = psX.tile([128, kn, 128], bf16, tag="xps")
                    for k in range(kn):
                        nc.tensor.transpose(
                            out=xps[:, k, :],
                            in_=gt[:, k0 + k, :],
                            identity=ident[:],
                        )
                    xsb = xpool.tile([128, kn, 128], bf16, tag="xsb")
                    do_copy(xsb[:], xps[:])
                    yps = psY.tile([128, kn, 80], f32, tag="yps")
                    for k in range(kn):
                        nc.tensor.matmul(
                            out=yps[:, k, :],
                            lhsT=xsb[:, k, :],
                            rhs=ksb[:, t * 80 : (t + 1) * 80],
                            start=True,
                            stop=True,
                        )
                    do_copy(ysb[:, k0 : k0 + kn, :], yps[:])
                nc.gpsimd.dma_scatter_add(
                    out_ap=tbl[q][:, :80],
                    in_ap=ysb[:],
                    idxs_ap=ssb[:, c0 * 8 : c0 * 8 + 8 * w],
                    num_idxs=ntok,
                    num_idxs_reg=ntok,
                    elem_size=80,
                    elem_step=128,
                    queue_num=q,
                )

            # interleave: sparse segments with center blocks spread between
            nseg = len(segments)
            cb = 0
            for i in range(nseg):
                emit_segment(i)
                want = (i + 1) * NBLK // nseg
                while cb < want:
                    emit_center(cb)
                    cb += 1
            while cb < NBLK:
                emit_center(cb)
                cb += 1

    print("tile build done", file=sys.stderr)
    nc.compile()
    print("bacc compile done", file=sys.stderr)
    return nc


_LAST = {"exec_time_ns": None, "results": None}


def kernel(feats, weight, w_sc0, w_sc1, coords):
    feats = np.ascontiguousarray(np.asarray(feats, np.float32))
    weight = np.asarray(weight, np.float32)
    w_sc0 = np.asarray(w_sc0, np.float32)
    w_sc1 = np.asarray(w_sc1, np.float32)
    coords = np.asarray(coords, np.int32)

    K = make_kernel_np(weight)
    K[CENTER_TAP] = K[CENTER_TAP] + w_sc_embed(w_sc0, w_sc1)
    ktaps = np.zeros((128, 125 * 80), np.float32)
    ktaps[:DIM] = K.transpose(1, 0, 2).reshape(DIM, 125 * 80)
    ktaps = np.ascontiguousarray(ktaps.astype(ml_dtypes.bfloat16))

    (
        feats_loc,
        feats_T,
        gidx_w,
        sidx_w,
        segments,
        seg_cols,
        W,
        order,
        SRC_ROWS,
    ) = build_plan(feats, coords)
    print(
        f"plan: W={W} nseg={len(segments)} SRC_ROWS={SRC_ROWS}",
        file=sys.stderr,
    )

    _install_axon_profile_hook()
    from concourse.bass_utils import run_bass_kernel_spmd

    nc = build_program(segments, seg_cols, W, SRC_ROWS)
    in_maps = [
        {
            "feats_loc": feats_loc[c],
            "feats_T": feats_T[c],
            "ktaps": ktaps,
            "gidx": gidx_w[c],
            "sidx": sidx_w[c],
        }
        for c in range(N_CORES)
    ]
    import os

    trace = os.environ.get("BASS_KERNEL_TRACE", "0") == "1"
    import time as _time

    res = None
    last_exc = None
    for attempt in range(4):
        try:
            res = run_bass_kernel_spmd(
                nc,
                in_maps,
                core_ids=list(range(N_CORES)),
                trace=trace and attempt == 0,
            )
            break
        except Exception as e:  # device flake: retry, later attempts untraced
            last_exc = e
            print(f"run attempt {attempt} failed: {e}", file=sys.stderr)
            _time.sleep(3.0)
    if res is None:
        raise last_exc
    print("hw run done", file=sys.stderr)
    _LAST["exec_time_ns"] = res.exec_time_ns
    _LAST["results"] = res
    out = np.empty((N, DIM), np.float32)
    for c in range(N_CORES):
        r = res.results[c]
        acc = np.asarray(r["out"])[:N_LOC, :DIM].astype(np.float32)
        for i in range(N_TBL):
            acc = acc + np.asarray(r[f"tbl{i}"])[:N_LOC, :DIM].astype(np.float32)
        out[order[c * N_LOC : (c + 1) * N_LOC]] = acc
    return out
